# revision 19
# baseline (speedup 1.0000x reference)
"""Cross multi-headed attention with relative position bias, sharded over 8 trn2 cores.

Sharding: query positions (T1=1024) split 128/core. pos_k (the 256MB tensor) is
indexed by query position, so each core streams a disjoint 1/8 slice exactly
once. K/V are computed from the full x on every core (cheap); outputs are
disjoint q-slices gathered on host.

Layout tricks:
  - pos_k is host-transposed/packed to (qgroup, dk+B, 4*T2) so it streams as
    the matmul moving operand (contraction dim dk on partitions) in 588KB DMAs.
  - The mask penalty is folded into the Bmat matmul as 8 extra contraction
    rows (one-hot per batch x -1e4 penalty): masking costs zero vector ops and
    masked attn weights are exactly 0 after exp underflow.
  - Bmat comes out of its matmul in per-q tiles (32 bh-rows x T2); a bf16 DRAM
    roundtrip regroups it into per-(b,h) tiles (128q x T2) which an
    identity-matmul accumulates straight onto the QK^T scores in PSUM.
  - LayerNorm gain/bias are folded into the projection weights on host;
    per-token mean/rstd are applied in natural layout (GPSIMD), then xn is
    PE-transposed for the projections.
  - Softmax denominators come from the exp activation's accum_out for free;
    no max-subtraction needed since scores are bounded (~N(0,2) after scale).
"""

import numpy as np
import ml_dtypes
from contextlib import ExitStack

import concourse.bass as bass
import concourse.tile as tile
import concourse.mybir as mybir
from concourse import bacc
from concourse.bass_utils import run_bass_kernel_spmd
from concourse.masks import make_identity

F32 = mybir.dt.float32
F32R = mybir.dt.float32r
BF16 = mybir.dt.bfloat16

B, T, F, H, DK = 8, 1024, 256, 4, 64
NCORES = 8
TQ = T // NCORES          # query rows per core (128)
TOK = B * T               # tokens for K/V (8192)
TOKQ = B * TQ             # query tokens per core (1024)
NBH = B * H               # 32 (b,h) pairs
KAUG = DK + B             # bmat contraction rows: 64 qk + 8 mask-penalty
EPS = 1e-5
PEN = 1e4                 # mask penalty (score -= PEN/8 => exp underflows to 0)

CFG = dict()

AF = mybir.ActivationFunctionType
ALU = mybir.AluOpType
AXX = mybir.AxisListType


def build_program(cfg=CFG, bv2_nz=False, bo_nz=False):
    nc = bacc.Bacc("TRN2", target_bir_lowering=False, debug=False)

    # ---- DRAM I/O ----
    # x packed as 16 super-tiles: [st, tok%128, (tok//128 % 4)*F + f]
    x_d = nc.dram_tensor("xp", [TOK // 512, 128, 4 * F], BF16, kind="ExternalInput")
    qt_d = nc.dram_tensor("q_t", [F, TOKQ], BF16, kind="ExternalInput")
    # pos_k packed 4 q per row-block: [qg, k, j*T + t]
    posk_d = nc.dram_tensor("posk_aug", [TQ // 4, KAUG, 4 * T], BF16,
                            kind="ExternalInput")
    oneh_d = nc.dram_tensor("onehot", [B, TQ * NBH], BF16, kind="ExternalInput")
    wq_d = nc.dram_tensor("wq", [F, F], BF16, kind="ExternalInput")
    wk_d = nc.dram_tensor("wk2", [F, F], BF16, kind="ExternalInput")
    wv_d = nc.dram_tensor("wv2", [F, F], BF16, kind="ExternalInput")
    wo_d = nc.dram_tensor("wo", [F, F], F32R, kind="ExternalInput")
    bq_d = nc.dram_tensor("bq_cols", [128, 2], F32, kind="ExternalInput")
    bk_d = nc.dram_tensor("bk2_cols", [128, 2], F32, kind="ExternalInput")
    bv_d = nc.dram_tensor("bv2_row", [1, F], BF16, kind="ExternalInput")
    bo_d = nc.dram_tensor("bo_row", [1, F], F32R, kind="ExternalInput")
    ones_d = nc.dram_tensor("ones_row", [1, 128], F32R, kind="ExternalInput")
    out_d = nc.dram_tensor("out", [B, TQ, F], F32, kind="ExternalOutput")

    with tile.TileContext(nc) as tc, ExitStack() as ctx:
        consts = ctx.enter_context(tc.tile_pool(name="consts", bufs=1))
        persist = ctx.enter_context(tc.tile_pool(name="persist", bufs=1))
        dram = ctx.enter_context(tc.tile_pool(name="dram", bufs=1, space="DRAM"))

        # ---- constants ----
        id_bf = consts.tile([128, 128], BF16)
        make_identity(nc, id_bf)
        id_f32 = consts.tile([128, 128], F32)
        make_identity(nc, id_f32)
        wq_sb = consts.tile([128, 2 * F], BF16)   # [f%128, (f//128)*F + n]
        wk_sb = consts.tile([128, 2 * F], BF16)
        wv_sb = consts.tile([128, 2 * F], BF16)
        wo_sb = consts.tile([64, 4 * F], F32R)    # [hd%64, (hd//64)*F + n]
        for w_sb, w_d in ((wq_sb, wq_d), (wk_sb, wk_d), (wv_sb, wv_d)):
            for fc in range(2):
                nc.sync.dma_start(w_sb[:, fc * F:(fc + 1) * F],
                                  w_d[fc * 128:(fc + 1) * 128, :])
        for ci in range(4):
            nc.sync.dma_start(wo_sb[:, ci * F:(ci + 1) * F],
                              wo_d[ci * 64:(ci + 1) * 64, :])
        bq_sb = consts.tile([128, 2], F32)
        nc.sync.dma_start(bq_sb[:], bq_d[:])
        bk_sb = consts.tile([128, 2], F32)
        nc.sync.dma_start(bk_sb[:], bk_d[:])
        if bv2_nz:
            bv_sb = consts.tile([1, F], BF16)
            nc.sync.dma_start(bv_sb[:], bv_d[:])
            ones_bf = consts.tile([1, 128], BF16)
            nc.gpsimd.memset(ones_bf[:], 1.0)
        if bo_nz:
            bo_sb = consts.tile([1, F], F32R)
            nc.sync.dma_start(bo_sb[:], bo_d[:])
            ones_r = consts.tile([1, 128], F32R)
            nc.sync.dma_start(ones_r[:], ones_d[:])
        eps_col = consts.tile([128, 1], F32)
        nc.gpsimd.memset(eps_col[:], EPS)

        # ---- persistent activations ----
        xnt_sb = persist.tile([128, 2 * TOK], BF16)    # xn^T: [f%128, (f//128)*TOK+tok]
        kt_sb = persist.tile([128, 2 * TOK], BF16)     # K^T:  [n%128, (n//128)*TOK+tok]
        v_sb = persist.tile([128, (TOK // 128) * F], BF16)  # V: [tok%128,(tok//128)*F+hd]
        qq_sb = persist.tile([KAUG, TQ * NBH], BF16)   # [d|b', b*H*TQ + h*TQ + q]
        qq_dup = persist.tile([128, TQ * NBH], BF16)   # rows 64:128 = qq_sb[0:64]
        nc.sync.dma_start(qq_sb[DK:KAUG, :], oneh_d[:])
        bmat_dram = dram.tile([NBH, TQ, T], BF16)

        NST = TOK // 512  # 16 super tiles
        SPG = 4           # super tiles per stats group

        # ================= Phase B: Q^T projection -> qq_sb =================
        with tc.tile_pool(name="qstage", bufs=1) as qstage, \
             tc.tile_pool(name="ps_q", bufs=2, space="PSUM") as ps_q:
            qt_tiles = []
            for fc in range(2):
                qt = qstage.tile([128, TOKQ], BF16, tag=f"qt{fc}")
                nc.sync.dma_start(qt[:], qt_d[fc * 128:(fc + 1) * 128, :])
                qt_tiles.append(qt)
            qv = qq_sb[0:DK].rearrange("p (b h q) -> p b q h", h=H, q=TQ)
            for h in range(H):
                qp = ps_q.tile([DK, TOKQ], F32, tag="qp")
                for half in range(2):
                    for fc in range(2):
                        nc.tensor.matmul(
                            qp[:, half * 512:(half + 1) * 512],
                            wq_sb[:, fc * F + h * DK: fc * F + (h + 1) * DK],
                            qt_tiles[fc][:, half * 512:(half + 1) * 512],
                            start=(fc == 0), stop=(fc == 1))
                src = qp.rearrange("p (b q) -> p b q", b=B)
                bias = bq_sb[64 * (h % 2):64 * (h % 2) + 64, h // 2: h // 2 + 1]
                nc.scalar.activation(qv[:, :, :, h], src, AF.Identity, bias=bias)
            nc.sync.dma_start(qq_dup[64:128, :], qq_sb[0:DK, :])

        # ===== Phases A+C+D interleaved: LN/xn^T, K^T/V proj, Bmat stream =====
        with tc.tile_pool(name="xstage", bufs=4) as xstage, \
             tc.tile_pool(name="stats", bufs=3) as stats, \
             tc.tile_pool(name="scratch", bufs=3) as scratch, \
             tc.tile_pool(name="ps_a", bufs=2, space="PSUM") as ps_a, \
             tc.tile_pool(name="ps_kv", bufs=2, space="PSUM") as ps_kv, \
             tc.tile_pool(name="poskst", bufs=6) as poskst, \
             tc.tile_pool(name="bmstage", bufs=4) as bmstage, \
             tc.tile_pool(name="ps_bm", bufs=2, space="PSUM") as ps_bm:

            def a_supertile(st):
                # 512 tokens: load, stats, affine, transpose into xnt
                GRP = 4
                xt = xstage.tile([128, 4 * F], BF16, tag="xt")
                nc.gpsimd.dma_start(xt[:], x_d[st])
                sums = stats.tile([128, GRP], F32, tag="sums")
                sumsq = stats.tile([128, GRP], F32, tag="sumsq")
                for c in range(4):
                    nc.vector.reduce_sum(sums[:, c:c + 1],
                                         xt[:, c * F:(c + 1) * F], axis=AXX.X)
                    sq = scratch.tile([128, F], BF16, tag="sq")
                    nc.scalar.activation(sq[:], xt[:, c * F:(c + 1) * F],
                                         AF.Square, accum_out=sumsq[:, c:c + 1])
                negmu = stats.tile([128, GRP], F32, tag="negmu")
                nc.vector.tensor_scalar(out=negmu[:], in0=sums[:], scalar1=-1.0 / F,
                                        scalar2=None, op0=ALU.mult)
                msq = stats.tile([128, GRP], F32, tag="msq")
                nc.vector.tensor_mul(msq[:], negmu[:], negmu[:])
                var = stats.tile([128, GRP], F32, tag="var")
                nc.vector.tensor_scalar(out=var[:], in0=sumsq[:], scalar1=1.0 / F,
                                        scalar2=None, op0=ALU.mult)
                nc.vector.tensor_tensor(var[:], var[:], msq[:], op=ALU.subtract)
                sig = stats.tile([128, GRP], F32, tag="sig")
                nc.scalar.activation(sig[:], var[:], AF.Sqrt, bias=eps_col[:])
                rsig = stats.tile([128, GRP], F32, tag="rsig")
                nc.vector.reciprocal(rsig[:], sig[:])
                cc = stats.tile([128, GRP], F32, tag="cc")
                nc.vector.tensor_mul(cc[:], negmu[:], rsig[:])
                xn = scratch.tile([128, 4 * F], BF16, tag="xn")
                for c in range(4):
                    nc.gpsimd.tensor_scalar(
                        out=xn[:, c * F:(c + 1) * F],
                        in0=xt[:, c * F:(c + 1) * F],
                        scalar1=rsig[:, c:c + 1], scalar2=cc[:, c:c + 1],
                        op0=ALU.mult, op1=ALU.add)
                for fc in range(2):
                    tp = ps_a.tile([128, 512], BF16, tag="tp")
                    for c in range(4):
                        nc.tensor.transpose(
                            tp[:, c * 128:(c + 1) * 128],
                            xn[:, c * F + fc * 128: c * F + fc * 128 + 128],
                            id_bf[:])
                    nc.vector.tensor_copy(
                        xnt_sb[:, fc * TOK + st * 512: fc * TOK + (st + 1) * 512],
                        tp[:])

            def kv_chunk(s):
                # K^T for token chunk s (512 tokens), V for the same 4 tiles
                for ncI in range(2):
                    kp = ps_kv.tile([128, 512], F32, tag="kv")
                    for fc in range(2):
                        nc.tensor.matmul(
                            kp[:],
                            wk_sb[:, fc * F + ncI * 128: fc * F + (ncI + 1) * 128],
                            xnt_sb[:, fc * TOK + s * 512: fc * TOK + (s + 1) * 512],
                            start=(fc == 0), stop=(fc == 1))
                    nc.vector.tensor_scalar(
                        out=kt_sb[:, ncI * TOK + s * 512: ncI * TOK + (s + 1) * 512],
                        in0=kp[:], scalar1=bk_sb[:, ncI:ncI + 1], scalar2=None,
                        op0=ALU.add)
                for gp in range(2):  # 2 token-tiles per V psum
                    vp = ps_kv.tile([128, 2 * F], F32, tag="kv")
                    for gi in range(2):
                        g = s * 4 + gp * 2 + gi
                        for fc in range(2):
                            nc.tensor.matmul(
                                vp[:, gi * F:(gi + 1) * F],
                                xnt_sb[:, fc * TOK + g * 128: fc * TOK + (g + 1) * 128],
                                wv_sb[:, fc * F:(fc + 1) * F],
                                start=(fc == 0), stop=(fc == 1 and not bv2_nz))
                        if bv2_nz:
                            nc.tensor.matmul(vp[:, gi * F:(gi + 1) * F],
                                             ones_bf[:], bv_sb[:],
                                             start=False, stop=True)
                    g0 = s * 4 + gp * 2
                    nc.vector.tensor_copy(v_sb[:, g0 * F:(g0 + 2) * F], vp[:])

            def bmat_group(qg):
                bp = ps_bm.tile([128, T], F32, tag="bp")
                pk = poskst.tile([KAUG, 4 * T], BF16, tag="pk")
                eng = (nc.sync, nc.scalar, nc.gpsimd)[qg % 3]
                eng.dma_start(pk[:], posk_d[qg])
                qq_v = qq_sb.rearrange("p (b h q) -> p b h q", h=H, q=TQ)
                for j in range(4):
                    q = qg * 4 + j
                    lhs = qq_v[:, :, :, q]
                    for half in range(2):
                        nc.tensor.matmul(
                            bp[32 * j:32 * (j + 1), half * 512:(half + 1) * 512],
                            lhs, pk[:, j * T + half * 512: j * T + (half + 1) * 512],
                            start=True, stop=True, tile_position=(0, 32 * j))
                bs = bmstage.tile([128, T], BF16, tag="bs")
                if qg % 3 == 2:
                    nc.vector.tensor_copy(bs[:], bp[:])
                else:
                    nc.scalar.activation(bs[:], bp[:], AF.Copy)
                dst = bmat_dram.rearrange("bh (qg j) t -> qg j bh t", j=4)[qg]
                nc.sync.dma_start(dst, bs[:])

            # interleave: 16 super-tiles / kv chunks, 32 bmat groups
            for s in range(NST):
                bmat_group(2 * s)
                a_supertile(s)
                bmat_group(2 * s + 1)
                kv_chunk(s)

        # ================= Phase E: attention per (b, h) =================
        with tc.tile_pool(name="attst", bufs=4) as attst, \
             tc.tile_pool(name="ps_s", bufs=2, space="PSUM") as ps_s, \
             tc.tile_pool(name="ps_at", bufs=2, space="PSUM") as ps_at, \
             tc.tile_pool(name="ps_small", bufs=2, space="PSUM") as ps_small, \
             tc.tile_pool(name="avout", bufs=2) as avout:
            qa0 = qq_sb[0:DK].rearrange("p (c q) -> p c q", c=NBH)
            qa1 = qq_dup[64:128].rearrange("p (c q) -> p c q", c=NBH)
            for b in range(B):
                avt = avout.tile([64, 4 * TQ], F32R, tag="avt")  # [hd%64,(hd//64)*TQ+q]
                for h in range(H):
                    bh = H * b + h
                    bm = attst.tile([128, T], BF16, tag="bm")
                    (nc.sync if bh % 2 == 0 else nc.gpsimd).dma_start(
                        bm[:], bmat_dram[bh])
                    lhsq = (qa0 if h % 2 == 0 else qa1)[:, bh, :]
                    sp = ps_s.tile([128, T], F32, tag="sp")
                    for half in range(2):
                        nc.tensor.matmul(
                            sp[:, half * 512:(half + 1) * 512], lhsq,
                            kt_sb[(h % 2) * 64:(h % 2) * 64 + 64,
                                  (h // 2) * TOK + b * T + half * 512:
                                  (h // 2) * TOK + b * T + (half + 1) * 512],
                            start=True, stop=False)
                        nc.tensor.matmul(
                            sp[:, half * 512:(half + 1) * 512], id_bf[:],
                            bm[:, half * 512:(half + 1) * 512],
                            start=False, stop=True)
                    attn = attst.tile([128, T], BF16, tag="attn")
                    den2 = attst.tile([128, 2], F32, tag="den2")
                    for half in range(2):
                        hs = slice(half * 512, (half + 1) * 512)
                        nc.scalar.activation(attn[:, hs], sp[:, hs], AF.Exp,
                                             scale=1.0 / np.sqrt(DK),
                                             accum_out=den2[:, half:half + 1])
                    dn = attst.tile([128, 1], F32, tag="dn")
                    nc.vector.tensor_add(dn[:], den2[:, 0:1], den2[:, 1:2])
                    rn = attst.tile([128, 1], F32, tag="rn")
                    nc.vector.reciprocal(rn[:], dn[:])
                    att = attst.tile([128, T], BF16, tag="att")  # attn^T
                    tp2 = ps_at.tile([128, T], BF16, tag="tp2")
                    for c in range(8):
                        nc.tensor.transpose(tp2[:, c * 128:(c + 1) * 128],
                                            attn[:, c * 128:(c + 1) * 128], id_bf[:])
                    nc.vector.tensor_copy(att[:], tp2[:])
                    avp = ps_small.tile([128, DK], F32, tag="small")
                    for c in range(8):
                        nc.tensor.matmul(
                            avp[:], att[:, c * 128:(c + 1) * 128],
                            v_sb[:, (b * 8 + c) * F + h * DK:
                                 (b * 8 + c) * F + (h + 1) * DK],
                            start=(c == 0), stop=(c == 7))
                    av = avout.tile([128, DK], F32, tag="av")
                    nc.vector.tensor_scalar(out=av[:], in0=avp[:], scalar1=rn[:],
                                            scalar2=None, op0=ALU.mult)
                    atp = ps_small.tile([DK, 128], F32, tag="small")
                    nc.tensor.transpose(atp[:], av[:], id_f32[:])
                    nc.scalar.activation(avt[:, h * TQ:(h + 1) * TQ], atp[:], AF.Copy)
                op = ps_small.tile([128, F], F32, tag="small")
                for ci in range(4):
                    nc.tensor.matmul(
                        op[:], avt[:, ci * TQ:(ci + 1) * TQ],
                        wo_sb[:, ci * F:(ci + 1) * F],
                        start=(ci == 0), stop=(ci == 3 and not bo_nz))
                if bo_nz:
                    nc.tensor.matmul(op[:], ones_r[:], bo_sb[:],
                                     start=False, stop=True)
                ob = avout.tile([128, F], F32, tag="ob")
                nc.vector.tensor_copy(ob[:], op[:])
                nc.sync.dma_start(out_d[b], ob[:])

    nc._dbg_names = {
        "xnt": xnt_sb.tensor.name, "kt": kt_sb.tensor.name,
        "v": v_sb.tensor.name, "qq": qq_sb.tensor.name,
        "bmat": bmat_dram.tensor.name,
    }
    nc.compile()
    return nc


def make_core_inputs(inputs, cfg=CFG):
    """Host-side sharding/layout. Returns (per_core_maps, bias_flags)."""
    x = np.asarray(inputs["x"], np.float32)
    q_in = np.asarray(inputs["q_in"], np.float32)
    pos_k = np.asarray(inputs["pos_k"], np.float32)
    mask = np.asarray(inputs["mask"])
    ln_g = np.asarray(inputs["ln_g"], np.float32)
    ln_b = np.asarray(inputs["ln_b"], np.float32)
    Wq, bq = np.asarray(inputs["Wq"], np.float32), np.asarray(inputs["bq"], np.float32)
    Wk, bk = np.asarray(inputs["Wk"], np.float32), np.asarray(inputs["bk"], np.float32)
    Wv, bv = np.asarray(inputs["Wv"], np.float32), np.asarray(inputs["bv"], np.float32)
    Wo, bo = np.asarray(inputs["Wo"], np.float32), np.asarray(inputs["bo"], np.float32)

    bf = ml_dtypes.bfloat16
    Wk2 = ln_g[:, None] * Wk
    bk2 = ln_b @ Wk + bk
    Wv2 = ln_g[:, None] * Wv
    bv2 = ln_b @ Wv + bv

    xp = np.ascontiguousarray(
        x.reshape(TOK // 512, 4, 128, F).transpose(0, 2, 1, 3).reshape(
            TOK // 512, 128, 4 * F)).astype(bf)
    shared = {
        "xp": xp,
        "wq": Wq.astype(bf),
        "wk2": Wk2.astype(bf),
        "wv2": Wv2.astype(bf),
        "wo": Wo.astype(np.float32),
        "bq_cols": np.ascontiguousarray(bq.reshape(2, 128).T).astype(np.float32),
        "bk2_cols": np.ascontiguousarray(bk2.reshape(2, 128).T).astype(np.float32),
        "bv2_row": bv2.reshape(1, F).astype(bf),
        "bo_row": bo.reshape(1, F).astype(np.float32),
        "ones_row": np.ones((1, 128), np.float32),
    }
    # onehot penalty rows: qq_aug[DK+b', b*H*TQ + h*TQ + q] = -PEN iff b'==b
    oh = np.zeros((B, B, H * TQ), np.float32)
    for bb in range(B):
        oh[bb, bb, :] = -PEN
    shared["onehot"] = np.ascontiguousarray(oh.reshape(B, NBH * TQ)).astype(bf)

    per_core = []
    for c in range(NCORES):
        qs = slice(c * TQ, (c + 1) * TQ)
        pa = np.empty((TQ, KAUG, T), bf)
        pa[:, :DK, :] = pos_k[qs].transpose(0, 2, 1).astype(bf)
        pa[:, DK:, :] = (1.0 - mask[:, qs, :].astype(np.float32)
                         ).transpose(1, 0, 2).astype(bf)
        # pack 4 q per row-block: [qg, k, j*T + t]
        pa = np.ascontiguousarray(
            pa.reshape(TQ // 4, 4, KAUG, T).transpose(0, 2, 1, 3).reshape(
                TQ // 4, KAUG, 4 * T))
        qt = np.ascontiguousarray(q_in[:, qs, :].reshape(TOKQ, F).T).astype(bf)
        m = dict(shared)
        m["posk_aug"] = pa
        m["q_t"] = qt
        per_core.append(m)
    flags = dict(bv2_nz=bool(np.any(bv2)), bo_nz=bool(np.any(bo)))
    return per_core, flags


_PROGRAM_CACHE = {}


def kernel(**inputs):
    per_core, flags = make_core_inputs(inputs, CFG)
    key = (tuple(sorted(CFG.items())), tuple(sorted(flags.items())))
    if key not in _PROGRAM_CACHE:
        _PROGRAM_CACHE[key] = build_program(CFG, **flags)
    nc = _PROGRAM_CACHE[key]
    res = run_bass_kernel_spmd(nc, per_core, core_ids=list(range(NCORES)))
    outs = [res.results[c]["out"] for c in range(NCORES)]
    return np.concatenate(outs, axis=1).astype(np.float32)


# revision 20
# speedup vs baseline: 1.2047x; 1.2047x over previous
"""Cross multi-headed attention with relative position bias, sharded over 8 trn2 cores.

Sharding: query positions (T1=1024) split 128/core. pos_k (the 256MB tensor) is
indexed by query position, so each core streams a disjoint 1/8 slice exactly
once. K/V are computed from the full x on every core (cheap); outputs are
disjoint q-slices gathered on host.

Layout tricks:
  - pos_k is host-transposed/packed to (qgroup, dk+B, 4*T2) so it streams as
    the matmul moving operand (contraction dim dk on partitions) in 588KB DMAs.
  - The mask penalty is folded into the Bmat matmul as 8 extra contraction
    rows (one-hot per batch x -1e4 penalty): masking costs zero vector ops and
    masked attn weights are exactly 0 after exp underflow.
  - Bmat comes out of its matmul in per-q tiles (32 bh-rows x T2); a bf16 DRAM
    roundtrip regroups it into per-(b,h) tiles (128q x T2) which an
    identity-matmul accumulates straight onto the QK^T scores in PSUM.
  - LayerNorm gain/bias are folded into the projection weights on host;
    per-token mean/rstd are applied in natural layout (GPSIMD), then xn is
    PE-transposed for the projections.
  - Softmax denominators come from the exp activation's accum_out for free;
    no max-subtraction needed since scores are bounded (~N(0,2) after scale).
"""

import numpy as np
import ml_dtypes
from contextlib import ExitStack

import concourse.bass as bass
import concourse.tile as tile
import concourse.mybir as mybir
from concourse import bacc
from concourse.bass_utils import run_bass_kernel_spmd
from concourse.masks import make_identity

F32 = mybir.dt.float32
F32R = mybir.dt.float32r
BF16 = mybir.dt.bfloat16

B, T, F, H, DK = 8, 1024, 256, 4, 64
NCORES = 8
TQ = T // NCORES          # query rows per core (128)
TOK = B * T               # tokens for K/V (8192)
TOKQ = B * TQ             # query tokens per core (1024)
NBH = B * H               # 32 (b,h) pairs
KAUG = DK + B             # bmat contraction rows: 64 qk + 8 mask-penalty
EPS = 1e-5
PEN = 1e4                 # mask penalty (score -= PEN/8 => exp underflows to 0)

CFG = dict()

AF = mybir.ActivationFunctionType
ALU = mybir.AluOpType
AXX = mybir.AxisListType


def build_program(cfg=CFG, bv2_nz=False, bo_nz=False):
    nc = bacc.Bacc("TRN2", target_bir_lowering=False, debug=False)

    # ---- DRAM I/O ----
    # x packed as 16 super-tiles: [st, tok%128, (tok//128 % 4)*F + f]
    x_d = nc.dram_tensor("xp", [TOK // 512, 128, 4 * F], BF16, kind="ExternalInput")
    qt_d = nc.dram_tensor("q_t", [F, TOKQ], BF16, kind="ExternalInput")
    # pos_k packed 4 q per row-block: [qg, k, j*T + t]
    posk_d = nc.dram_tensor("posk_aug", [TQ // 2, KAUG, 2 * T], BF16,
                            kind="ExternalInput")
    oneh_d = nc.dram_tensor("onehot", [B, TQ * NBH], BF16, kind="ExternalInput")
    wq_d = nc.dram_tensor("wq", [F, F], BF16, kind="ExternalInput")
    wk_d = nc.dram_tensor("wk2", [F, F], BF16, kind="ExternalInput")
    wv_d = nc.dram_tensor("wv2", [F, F], BF16, kind="ExternalInput")
    wo_d = nc.dram_tensor("wo", [F, F], F32R, kind="ExternalInput")
    bq_d = nc.dram_tensor("bq_cols", [128, 2], F32, kind="ExternalInput")
    bk_d = nc.dram_tensor("bk2_cols", [128, 2], F32, kind="ExternalInput")
    bv_d = nc.dram_tensor("bv2_row", [1, F], BF16, kind="ExternalInput")
    bo_d = nc.dram_tensor("bo_row", [1, F], F32R, kind="ExternalInput")
    ones_d = nc.dram_tensor("ones_row", [1, 128], F32R, kind="ExternalInput")
    out_d = nc.dram_tensor("out", [B, TQ, F], F32, kind="ExternalOutput")

    with tile.TileContext(nc) as tc, ExitStack() as ctx:
        consts = ctx.enter_context(tc.tile_pool(name="consts", bufs=1))
        persist = ctx.enter_context(tc.tile_pool(name="persist", bufs=1))
        dram = ctx.enter_context(tc.tile_pool(name="dram", bufs=1, space="DRAM"))

        # ---- constants ----
        id_bf = consts.tile([128, 128], BF16)
        make_identity(nc, id_bf)
        id_f32 = consts.tile([128, 128], F32)
        make_identity(nc, id_f32)
        wq_sb = consts.tile([128, 2 * F], BF16)   # [f%128, (f//128)*F + n]
        wk_sb = consts.tile([128, 2 * F], BF16)
        wv_sb = consts.tile([128, 2 * F], BF16)
        wo_sb = consts.tile([64, 4 * F], F32R)    # [hd%64, (hd//64)*F + n]
        for w_sb, w_d in ((wq_sb, wq_d), (wk_sb, wk_d), (wv_sb, wv_d)):
            for fc in range(2):
                nc.sync.dma_start(w_sb[:, fc * F:(fc + 1) * F],
                                  w_d[fc * 128:(fc + 1) * 128, :])
        for ci in range(4):
            nc.sync.dma_start(wo_sb[:, ci * F:(ci + 1) * F],
                              wo_d[ci * 64:(ci + 1) * 64, :])
        bq_sb = consts.tile([128, 2], F32)
        nc.sync.dma_start(bq_sb[:], bq_d[:])
        bk_sb = consts.tile([128, 2], F32)
        nc.sync.dma_start(bk_sb[:], bk_d[:])
        if bv2_nz:
            bv_sb = consts.tile([1, F], BF16)
            nc.sync.dma_start(bv_sb[:], bv_d[:])
            ones_bf = consts.tile([1, 128], BF16)
            nc.gpsimd.memset(ones_bf[:], 1.0)
        if bo_nz:
            bo_sb = consts.tile([1, F], F32R)
            nc.sync.dma_start(bo_sb[:], bo_d[:])
            ones_r = consts.tile([1, 128], F32R)
            nc.sync.dma_start(ones_r[:], ones_d[:])
        eps_col = consts.tile([128, 1], F32)
        nc.gpsimd.memset(eps_col[:], EPS)

        # ---- persistent activations ----
        xnt_sb = persist.tile([128, 2 * TOK], BF16)    # xn^T: [f%128, (f//128)*TOK+tok]
        kt_sb = persist.tile([128, 2 * TOK], BF16)     # K^T:  [n%128, (n//128)*TOK+tok]
        v_sb = persist.tile([128, (TOK // 128) * F], BF16)  # V: [tok%128,(tok//128)*F+hd]
        qq_sb = persist.tile([KAUG, TQ * NBH], BF16)   # [d|b', b*H*TQ + h*TQ + q]
        qq_dup = persist.tile([128, TQ * NBH], BF16)   # rows 64:128 = qq_sb[0:64]
        nc.sync.dma_start(qq_sb[DK:KAUG, :], oneh_d[:])
        bmat_dram = dram.tile([NBH, TQ, T], BF16)

        NST = TOK // 512  # 16 super tiles
        SPG = 4           # super tiles per stats group

        # ================= Phase B: Q^T projection -> qq_sb =================
        with tc.tile_pool(name="qstage", bufs=1) as qstage, \
             tc.tile_pool(name="ps_q", bufs=2, space="PSUM") as ps_q:
            qt_tiles = []
            for fc in range(2):
                qt = qstage.tile([128, TOKQ], BF16, tag=f"qt{fc}")
                nc.sync.dma_start(qt[:], qt_d[fc * 128:(fc + 1) * 128, :])
                qt_tiles.append(qt)
            qv = qq_sb[0:DK].rearrange("p (b h q) -> p b q h", h=H, q=TQ)
            for h in range(H):
                qp = ps_q.tile([DK, TOKQ], F32, tag="qp")
                for half in range(2):
                    for fc in range(2):
                        nc.tensor.matmul(
                            qp[:, half * 512:(half + 1) * 512],
                            wq_sb[:, fc * F + h * DK: fc * F + (h + 1) * DK],
                            qt_tiles[fc][:, half * 512:(half + 1) * 512],
                            start=(fc == 0), stop=(fc == 1))
                src = qp.rearrange("p (b q) -> p b q", b=B)
                bias = bq_sb[64 * (h % 2):64 * (h % 2) + 64, h // 2: h // 2 + 1]
                nc.scalar.activation(qv[:, :, :, h], src, AF.Identity, bias=bias)
            nc.sync.dma_start(qq_dup[64:128, :], qq_sb[0:DK, :])

        # ===== Phases A+C+D interleaved: LN/xn^T, K^T/V proj, Bmat stream =====
        with tc.tile_pool(name="xstage", bufs=4) as xstage, \
             tc.tile_pool(name="stats", bufs=3) as stats, \
             tc.tile_pool(name="scratch", bufs=3) as scratch, \
             tc.tile_pool(name="ps_a", bufs=2, space="PSUM") as ps_a, \
             tc.tile_pool(name="ps_kv", bufs=2, space="PSUM") as ps_kv, \
             tc.tile_pool(name="poskst", bufs=10) as poskst, \
             tc.tile_pool(name="bmstage", bufs=4) as bmstage, \
             tc.tile_pool(name="ps_bm", bufs=2, space="PSUM") as ps_bm:

            def a_supertile(st):
                # 512 tokens: load, stats, affine, transpose into xnt
                GRP = 4
                xt = xstage.tile([128, 4 * F], BF16, tag="xt")
                nc.sync.dma_start(xt[:], x_d[st])
                sums = stats.tile([128, GRP], F32, tag="sums")
                sumsq = stats.tile([128, GRP], F32, tag="sumsq")
                for c in range(4):
                    nc.vector.reduce_sum(sums[:, c:c + 1],
                                         xt[:, c * F:(c + 1) * F], axis=AXX.X)
                    sq = scratch.tile([128, F], BF16, tag="sq")
                    nc.scalar.activation(sq[:], xt[:, c * F:(c + 1) * F],
                                         AF.Square, accum_out=sumsq[:, c:c + 1])
                negmu = stats.tile([128, GRP], F32, tag="negmu")
                nc.vector.tensor_scalar(out=negmu[:], in0=sums[:], scalar1=-1.0 / F,
                                        scalar2=None, op0=ALU.mult)
                msq = stats.tile([128, GRP], F32, tag="msq")
                nc.vector.tensor_mul(msq[:], negmu[:], negmu[:])
                var = stats.tile([128, GRP], F32, tag="var")
                nc.vector.tensor_scalar(out=var[:], in0=sumsq[:], scalar1=1.0 / F,
                                        scalar2=None, op0=ALU.mult)
                nc.vector.tensor_tensor(var[:], var[:], msq[:], op=ALU.subtract)
                sig = stats.tile([128, GRP], F32, tag="sig")
                nc.scalar.activation(sig[:], var[:], AF.Sqrt, bias=eps_col[:])
                rsig = stats.tile([128, GRP], F32, tag="rsig")
                nc.vector.reciprocal(rsig[:], sig[:])
                cc = stats.tile([128, GRP], F32, tag="cc")
                nc.vector.tensor_mul(cc[:], negmu[:], rsig[:])
                xn = scratch.tile([128, 4 * F], BF16, tag="xn")
                for c in range(4):
                    nc.gpsimd.tensor_scalar(
                        out=xn[:, c * F:(c + 1) * F],
                        in0=xt[:, c * F:(c + 1) * F],
                        scalar1=rsig[:, c:c + 1], scalar2=cc[:, c:c + 1],
                        op0=ALU.mult, op1=ALU.add)
                for fc in range(2):
                    tp = ps_a.tile([128, 512], BF16, tag="tp")
                    for c in range(4):
                        nc.tensor.transpose(
                            tp[:, c * 128:(c + 1) * 128],
                            xn[:, c * F + fc * 128: c * F + fc * 128 + 128],
                            id_bf[:])
                    nc.vector.tensor_copy(
                        xnt_sb[:, fc * TOK + st * 512: fc * TOK + (st + 1) * 512],
                        tp[:])

            def kv_chunk(s):
                # K^T for token chunk s (512 tokens), V for the same 4 tiles
                for ncI in range(2):
                    kp = ps_kv.tile([128, 512], F32, tag="kv")
                    for fc in range(2):
                        nc.tensor.matmul(
                            kp[:],
                            wk_sb[:, fc * F + ncI * 128: fc * F + (ncI + 1) * 128],
                            xnt_sb[:, fc * TOK + s * 512: fc * TOK + (s + 1) * 512],
                            start=(fc == 0), stop=(fc == 1))
                    nc.vector.tensor_scalar(
                        out=kt_sb[:, ncI * TOK + s * 512: ncI * TOK + (s + 1) * 512],
                        in0=kp[:], scalar1=bk_sb[:, ncI:ncI + 1], scalar2=None,
                        op0=ALU.add)
                for gp in range(2):  # 2 token-tiles per V psum
                    vp = ps_kv.tile([128, 2 * F], F32, tag="kv")
                    for gi in range(2):
                        g = s * 4 + gp * 2 + gi
                        for fc in range(2):
                            nc.tensor.matmul(
                                vp[:, gi * F:(gi + 1) * F],
                                xnt_sb[:, fc * TOK + g * 128: fc * TOK + (g + 1) * 128],
                                wv_sb[:, fc * F:(fc + 1) * F],
                                start=(fc == 0), stop=(fc == 1 and not bv2_nz))
                        if bv2_nz:
                            nc.tensor.matmul(vp[:, gi * F:(gi + 1) * F],
                                             ones_bf[:], bv_sb[:],
                                             start=False, stop=True)
                    g0 = s * 4 + gp * 2
                    nc.vector.tensor_copy(v_sb[:, g0 * F:(g0 + 2) * F], vp[:])

            def bmat_group(qg):
                bp = ps_bm.tile([128, T], F32, tag="bp")
                qq_v = qq_sb.rearrange("p (b h q) -> p b h q", h=H, q=TQ)
                for jp in range(2):
                    pk = poskst.tile([KAUG, 2 * T], BF16, tag="pk")
                    eng = (nc.sync, nc.gpsimd, nc.scalar)[(2 * qg + jp) % 3]
                    eng.dma_start(pk[:], posk_d[2 * qg + jp])
                    for jj in range(2):
                        j = jp * 2 + jj
                        q = qg * 4 + j
                        lhs = qq_v[:, :, :, q]
                        for half in range(2):
                            nc.tensor.matmul(
                                bp[32 * j:32 * (j + 1), half * 512:(half + 1) * 512],
                                lhs,
                                pk[:, jj * T + half * 512: jj * T + (half + 1) * 512],
                                start=True, stop=True, tile_position=(0, 32 * j))
                bs = bmstage.tile([128, T], BF16, tag="bs")
                if qg % 3 == 2:
                    nc.vector.tensor_copy(bs[:], bp[:])
                else:
                    nc.scalar.activation(bs[:], bp[:], AF.Copy)
                dst = bmat_dram.rearrange("bh (qg j) t -> qg j bh t", j=4)[qg]
                nc.gpsimd.dma_start(dst, bs[:])

            # interleave: 16 super-tiles / kv chunks, 32 bmat groups
            for s in range(NST):
                a_supertile(s)
                bmat_group(2 * s)
                kv_chunk(s)
                bmat_group(2 * s + 1)

        # ================= Phase E: attention per (b, h) =================
        with tc.tile_pool(name="attst", bufs=4) as attst, \
             tc.tile_pool(name="ps_s", bufs=2, space="PSUM") as ps_s, \
             tc.tile_pool(name="ps_at", bufs=2, space="PSUM") as ps_at, \
             tc.tile_pool(name="ps_small", bufs=2, space="PSUM") as ps_small, \
             tc.tile_pool(name="avout", bufs=2) as avout:
            qa0 = qq_sb[0:DK].rearrange("p (c q) -> p c q", c=NBH)
            qa1 = qq_dup[64:128].rearrange("p (c q) -> p c q", c=NBH)
            for b in range(B):
                avt = avout.tile([64, 4 * TQ], F32R, tag="avt")  # [hd%64,(hd//64)*TQ+q]
                for h in range(H):
                    bh = H * b + h
                    bm = attst.tile([128, T], BF16, tag="bm")
                    (nc.sync if bh % 2 == 0 else nc.gpsimd).dma_start(
                        bm[:], bmat_dram[bh])
                    lhsq = (qa0 if h % 2 == 0 else qa1)[:, bh, :]
                    sp = ps_s.tile([128, T], F32, tag="sp")
                    for half in range(2):
                        nc.tensor.matmul(
                            sp[:, half * 512:(half + 1) * 512], lhsq,
                            kt_sb[(h % 2) * 64:(h % 2) * 64 + 64,
                                  (h // 2) * TOK + b * T + half * 512:
                                  (h // 2) * TOK + b * T + (half + 1) * 512],
                            start=True, stop=False)
                        nc.tensor.matmul(
                            sp[:, half * 512:(half + 1) * 512], id_bf[:],
                            bm[:, half * 512:(half + 1) * 512],
                            start=False, stop=True)
                    attn = attst.tile([128, T], BF16, tag="attn")
                    den2 = attst.tile([128, 2], F32, tag="den2")
                    for half in range(2):
                        hs = slice(half * 512, (half + 1) * 512)
                        nc.scalar.activation(attn[:, hs], sp[:, hs], AF.Exp,
                                             scale=1.0 / np.sqrt(DK),
                                             accum_out=den2[:, half:half + 1])
                    dn = attst.tile([128, 1], F32, tag="dn")
                    nc.vector.tensor_add(dn[:], den2[:, 0:1], den2[:, 1:2])
                    rn = attst.tile([128, 1], F32, tag="rn")
                    nc.vector.reciprocal(rn[:], dn[:])
                    att = attst.tile([128, T], BF16, tag="att")  # attn^T
                    tp2 = ps_at.tile([128, T], BF16, tag="tp2")
                    for c in range(8):
                        nc.tensor.transpose(tp2[:, c * 128:(c + 1) * 128],
                                            attn[:, c * 128:(c + 1) * 128], id_bf[:])
                    nc.vector.tensor_copy(att[:], tp2[:])
                    avp = ps_small.tile([128, DK], F32, tag="small")
                    for c in range(8):
                        nc.tensor.matmul(
                            avp[:], att[:, c * 128:(c + 1) * 128],
                            v_sb[:, (b * 8 + c) * F + h * DK:
                                 (b * 8 + c) * F + (h + 1) * DK],
                            start=(c == 0), stop=(c == 7))
                    av = avout.tile([128, DK], F32, tag="av")
                    nc.vector.tensor_scalar(out=av[:], in0=avp[:], scalar1=rn[:],
                                            scalar2=None, op0=ALU.mult)
                    atp = ps_small.tile([DK, 128], F32, tag="small")
                    nc.tensor.transpose(atp[:], av[:], id_f32[:])
                    nc.scalar.activation(avt[:, h * TQ:(h + 1) * TQ], atp[:], AF.Copy)
                op = ps_small.tile([128, F], F32, tag="small")
                for ci in range(4):
                    nc.tensor.matmul(
                        op[:], avt[:, ci * TQ:(ci + 1) * TQ],
                        wo_sb[:, ci * F:(ci + 1) * F],
                        start=(ci == 0), stop=(ci == 3 and not bo_nz))
                if bo_nz:
                    nc.tensor.matmul(op[:], ones_r[:], bo_sb[:],
                                     start=False, stop=True)
                ob = avout.tile([128, F], F32, tag="ob")
                nc.vector.tensor_copy(ob[:], op[:])
                nc.sync.dma_start(out_d[b], ob[:])

    nc._dbg_names = {
        "xnt": xnt_sb.tensor.name, "kt": kt_sb.tensor.name,
        "v": v_sb.tensor.name, "qq": qq_sb.tensor.name,
        "bmat": bmat_dram.tensor.name,
    }
    nc.compile()
    return nc


def make_core_inputs(inputs, cfg=CFG):
    """Host-side sharding/layout. Returns (per_core_maps, bias_flags)."""
    x = np.asarray(inputs["x"], np.float32)
    q_in = np.asarray(inputs["q_in"], np.float32)
    pos_k = np.asarray(inputs["pos_k"], np.float32)
    mask = np.asarray(inputs["mask"])
    ln_g = np.asarray(inputs["ln_g"], np.float32)
    ln_b = np.asarray(inputs["ln_b"], np.float32)
    Wq, bq = np.asarray(inputs["Wq"], np.float32), np.asarray(inputs["bq"], np.float32)
    Wk, bk = np.asarray(inputs["Wk"], np.float32), np.asarray(inputs["bk"], np.float32)
    Wv, bv = np.asarray(inputs["Wv"], np.float32), np.asarray(inputs["bv"], np.float32)
    Wo, bo = np.asarray(inputs["Wo"], np.float32), np.asarray(inputs["bo"], np.float32)

    bf = ml_dtypes.bfloat16
    Wk2 = ln_g[:, None] * Wk
    bk2 = ln_b @ Wk + bk
    Wv2 = ln_g[:, None] * Wv
    bv2 = ln_b @ Wv + bv

    xp = np.ascontiguousarray(
        x.reshape(TOK // 512, 4, 128, F).transpose(0, 2, 1, 3).reshape(
            TOK // 512, 128, 4 * F)).astype(bf)
    shared = {
        "xp": xp,
        "wq": Wq.astype(bf),
        "wk2": Wk2.astype(bf),
        "wv2": Wv2.astype(bf),
        "wo": Wo.astype(np.float32),
        "bq_cols": np.ascontiguousarray(bq.reshape(2, 128).T).astype(np.float32),
        "bk2_cols": np.ascontiguousarray(bk2.reshape(2, 128).T).astype(np.float32),
        "bv2_row": bv2.reshape(1, F).astype(bf),
        "bo_row": bo.reshape(1, F).astype(np.float32),
        "ones_row": np.ones((1, 128), np.float32),
    }
    # onehot penalty rows: qq_aug[DK+b', b*H*TQ + h*TQ + q] = -PEN iff b'==b
    oh = np.zeros((B, B, H * TQ), np.float32)
    for bb in range(B):
        oh[bb, bb, :] = -PEN
    shared["onehot"] = np.ascontiguousarray(oh.reshape(B, NBH * TQ)).astype(bf)

    per_core = []
    for c in range(NCORES):
        qs = slice(c * TQ, (c + 1) * TQ)
        pa = np.empty((TQ, KAUG, T), bf)
        pa[:, :DK, :] = pos_k[qs].transpose(0, 2, 1).astype(bf)
        pa[:, DK:, :] = (1.0 - mask[:, qs, :].astype(np.float32)
                         ).transpose(1, 0, 2).astype(bf)
        # pack 2 q per row-block: [qp, k, j*T + t]
        pa = np.ascontiguousarray(
            pa.reshape(TQ // 2, 2, KAUG, T).transpose(0, 2, 1, 3).reshape(
                TQ // 2, KAUG, 2 * T))
        qt = np.ascontiguousarray(q_in[:, qs, :].reshape(TOKQ, F).T).astype(bf)
        m = dict(shared)
        m["posk_aug"] = pa
        m["q_t"] = qt
        per_core.append(m)
    flags = dict(bv2_nz=bool(np.any(bv2)), bo_nz=bool(np.any(bo)))
    return per_core, flags


_PROGRAM_CACHE = {}


def kernel(**inputs):
    per_core, flags = make_core_inputs(inputs, CFG)
    key = (tuple(sorted(CFG.items())), tuple(sorted(flags.items())))
    if key not in _PROGRAM_CACHE:
        _PROGRAM_CACHE[key] = build_program(CFG, **flags)
    nc = _PROGRAM_CACHE[key]
    res = run_bass_kernel_spmd(nc, per_core, core_ids=list(range(NCORES)))
    outs = [res.results[c]["out"] for c in range(NCORES)]
    return np.concatenate(outs, axis=1).astype(np.float32)
